# revision 1
# baseline (speedup 1.0000x reference)
"""Trainium2 Bass kernel for nn_DecoderLSTM_noAttention (greedy decode LSTM).

Strategy (8 NeuronCores, SPMD, all-fp32 numerics):
- Vocab-sharded FC: each core holds a 4000-column slice of W_fc.T and
  computes its logits slice each step in 8 PSUM-bank-sized chunks; each chunk
  is DMA'd straight from PSUM to the output and fed to DVE max/max_index
  (overlapped under the remaining FC matmuls).
- The input-side gate contributions are precomputed on the host as
  gtab = embedding @ W_ih.T + (b_ih + b_hh)  [32000, 2048], so a step's
  gates need only one indirect row-gather plus the on-chip h @ W_hh.T.
  The W_hh matmul for step t+1 runs during step t's argmax exchange,
  keeping the PE warm.
- Greedy argmax: per-chunk top-8, on-chip combine (first-occurrence
  tie-breaks), then an AllGather of (best value, global id) per batch row;
  every core picks the global winner deterministically.
- Batch-sharded phase 0 for the encoder mean + AllGather of summaries.
- Output: each core writes logits [31, 64, 4000]; the host assembles the
  full [64, 32, 32000] (t=0 stays zero).
"""
import numpy as np

import concourse.bass as bass
import concourse.bacc as bacc
import concourse.tile as tile
from concourse import mybir
from concourse.bass_utils import run_bass_kernel_spmd
from concourse.masks import make_identity

F32 = mybir.dt.float32
F16 = mybir.dt.float16
I32 = mybir.dt.int32
U32 = mybir.dt.uint32
AF = mybir.ActivationFunctionType
OP = mybir.AluOpType

B = 64          # batch
H = 512         # hidden = embed
V = 32000       # vocab
T = 32          # max_len
NPIX = 196
NCORES = 8
BL = B // NCORES      # local batch (phase 0)
VL = V // NCORES      # local vocab slice
NSTEPS = T - 1
GD = 2048             # gate dim

# FC chunking: bank-sized
CHUNKS = [(q * 512, 512) for q in range(7)] + [(3584, 416)]
NQ = len(CHUNKS)

_CACHE = {}


def _build_nc(nsteps=NSTEPS, out_slots=NSTEPS, no_cc=False, no_max=False):
    nc = bacc.Bacc("TRN2", target_bir_lowering=False, debug=False, num_devices=NCORES)

    # ---- DRAM parameters ----
    gtab_d = nc.dram_tensor("gtab", [V, GD], F32, kind="ExternalInput")
    whh_d = nc.dram_tensor("whhT", [4, 128, GD], F32, kind="ExternalInput")
    wfchi_d = nc.dram_tensor("wfcThi", [4, 128, VL], F16, kind="ExternalInput")
    wfclo_d = nc.dram_tensor("wfcTlo", [4, 128, VL], F16, kind="ExternalInput")
    winit_d = nc.dram_tensor("winitT", [4, 128, 1024], F32, kind="ExternalInput")
    bfc_d = nc.dram_tensor("bfc", [1, VL], F32, kind="ExternalInput")
    binit_d = nc.dram_tensor("binit", [1, 1024], F32, kind="ExternalInput")
    enc_d = nc.dram_tensor("enc", [13, 128, H], F32, kind="ExternalInput")
    blk_d = nc.dram_tensor("blkdiag", [128, 13 * 8], F32, kind="ExternalInput")
    tok0_d = nc.dram_tensor("tok0", [B, 1], I32, kind="ExternalInput")
    vbase_d = nc.dram_tensor("vbase", [B, 1], F32, kind="ExternalInput")

    out_d = nc.dram_tensor("logits", [out_slots, B, VL], F32, kind="ExternalOutput")

    with tile.TileContext(nc) as tc:
        import contextlib
        with contextlib.ExitStack() as ctx:
            const = ctx.enter_context(tc.tile_pool(name="const", bufs=1))
            work = ctx.enter_context(tc.tile_pool(name="work", bufs=1))
            hc = ctx.enter_context(tc.tile_pool(name="hc", bufs=2))
            small = ctx.enter_context(tc.tile_pool(name="small", bufs=2))
            lgp = ctx.enter_context(tc.tile_pool(name="lgp", bufs=1))
            ptr = ctx.enter_context(tc.tile_pool(name="ptr", bufs=1, space="PSUM"))
            pg = ctx.enter_context(tc.tile_pool(name="pg", bufs=1, space="PSUM"))
            pfc = ctx.enter_context(tc.tile_pool(name="pfc", bufs=2, space="PSUM"))
            dram = ctx.enter_context(tc.tile_pool(name="dram", bufs=2, space="DRAM"))
            dramsh = ctx.enter_context(
                tc.tile_pool(name="dramsh", bufs=2, space="DRAM"))

            # ---- constants / weights into SBUF ----
            ident = const.tile([B, B], F32)
            make_identity(nc, ident[:])
            ones1 = const.tile([1, B], F32)
            nc.vector.memset(ones1[:], 1.0)
            vb64 = const.tile([B, 1], F32)
            nc.sync.dma_start(vb64[:], vbase_d[:])
            qbase = const.tile([B, NQ * 8], F32)
            for q, (off, _w) in enumerate(CHUNKS):
                nc.vector.memset(qbase[:, q * 8:(q + 1) * 8], float(off))

            whh = []
            for k in range(4):
                w = const.tile([128, GD], F32, tag=f"whh{k}")
                nc.sync.dma_start(w[:], whh_d[k])
                whh.append(w)
            wfchi, wfclo = [], []
            for k in range(4):
                w = const.tile([128, VL], F16, tag=f"wfchi{k}")
                nc.sync.dma_start(w[:], wfchi_d[k])
                wfchi.append(w)
                w = const.tile([128, VL], F16, tag=f"wfclo{k}")
                nc.sync.dma_start(w[:], wfclo_d[k])
                wfclo.append(w)
            bfc = const.tile([1, VL], F32)
            nc.sync.dma_start(bfc[:], bfc_d[:])
            binit = const.tile([1, 1024], F32)
            nc.sync.dma_start(binit[:], binit_d[:])
            blk = work.tile([128, 13 * 8], F32, tag="gx")
            nc.sync.dma_start(blk[:], blk_d[:])

            def transpose_to(src, dst_tile):
                """src: SBUF [B, 512] fp32 -> dst SBUF [128, 4*B] feature-major."""
                for k in range(4):
                    pt = ptr.tile([128, B], F32, tag="ptr")
                    nc.tensor.transpose(
                        out=pt[:], in_=src[:, k * 128:(k + 1) * 128],
                        identity=ident[:])
                    nc.scalar.copy(dst_tile[:, k * B:(k + 1) * B], pt[:])

            # ================= phase 0 =================
            psum0 = pg.tile([BL, H], F32, tag="pg")
            for k in range(13):
                et = work.tile([128, H], F32, tag="gsum")
                nc.sync.dma_start(et[:], enc_d[k])
                nc.tensor.matmul(
                    psum0[:], lhsT=blk[:, k * 8:(k + 1) * 8], rhs=et[:],
                    start=(k == 0), stop=(k == 12))
            sums = work.tile([BL, H], F32, tag="tng")
            nc.scalar.copy(sums[:], psum0[:])

            sumfull = work.tile([B, H], F32, tag="sgo")
            if no_cc:
                for ci in range(NCORES):
                    nc.vector.tensor_copy(sumfull[ci * BL:(ci + 1) * BL, :], sums[:])
            else:
                cc0_in = dram.tile([BL, H], F32, tag="cc0i")
                cc0_out = dramsh.tile([NCORES, BL, H], F32, addr_space="Shared",
                                      tag="cc0o")
                nc.sync.dma_start(cc0_in[:], sums[:])
                nc.gpsimd.collective_compute(
                    "AllGather", OP.bypass,
                    replica_groups=[list(range(NCORES))],
                    ins=[cc0_in[:]], outs=[cc0_out[:]])
                nc.sync.dma_start(sumfull[:], cc0_out[:].rearrange("c b h -> (c b) h"))

            sumT = work.tile([128, 4 * B], F32, tag="hT")
            transpose_to(sumfull, sumT)

            for n in range(2):
                ph = pfc.tile([B, 512], F32, tag="pfc")
                for k in range(4):
                    wi = work.tile([128, 1024], F32, tag="sigif")
                    nc.sync.dma_start(wi[:], winit_d[k])
                    nc.tensor.matmul(
                        ph[:], lhsT=sumT[:, k * B:(k + 1) * B],
                        rhs=wi[:, n * 512:(n + 1) * 512],
                        start=(k == 0), stop=False)
                nc.tensor.matmul(
                    ph[:], lhsT=ones1[:],
                    rhs=binit[:, n * 512:(n + 1) * 512],
                    start=False, stop=True)
                dst = hc.tile([B, H], F32, tag=("h" if n == 0 else "c"))
                nc.scalar.copy(dst[:], ph[:])
                if n == 0:
                    h_cur = dst
                else:
                    c_cur = dst
            hT = work.tile([128, 4 * B], F32, tag="hT")
            transpose_to(h_cur, hT)
            hhiT = work.tile([128, 4 * B], F16, tag="hhiT")
            nc.vector.tensor_copy(hhiT[:], hT[:])
            hres = work.tile([128, 4 * B], F32, tag="hres")
            nc.vector.tensor_sub(hres[:], hT[:], hhiT[:])
            hloT = work.tile([128, 4 * B], F16, tag="hloT")
            nc.vector.tensor_copy(hloT[:], hres[:])

            tok = small.tile([B, 1], I32, tag="tok")
            nc.sync.dma_start(tok[:], tok0_d[:])

            # ================= decode steps =================
            for t in range(nsteps):
                # --- gather input-gate contributions (gtab rows) ---
                gx = work.tile([B, GD], F32, tag="gx")
                nc.gpsimd.indirect_dma_start(
                    out=gx[:], out_offset=None, in_=gtab_d[:],
                    in_offset=bass.IndirectOffsetOnAxis(ap=tok[:, :1], axis=0))

                # --- h @ W_hh.T into PSUM (ready early: overlaps exchange) ---
                pgt = pg.tile([B, GD], F32, tag="pg")
                for k in range(4):
                    for n in range(4):
                        nc.tensor.matmul(
                            pgt[:, n * 512:(n + 1) * 512],
                            lhsT=hT[:, k * B:(k + 1) * B],
                            rhs=whh[k][:, n * 512:(n + 1) * 512],
                            start=(k == 0), stop=(k == 3))

                # --- gates = psum + gx; pointwise LSTM ---
                gsum = work.tile([B, GD], F32, tag="gsum")
                nc.vector.tensor_add(gsum[:], pgt[:], gx[:])
                sig_if = work.tile([B, 1024], F32, tag="sigif")
                nc.scalar.activation(sig_if[:], gsum[:, 0:1024], AF.Sigmoid)
                tng = work.tile([B, 512], F32, tag="tng")
                nc.scalar.activation(tng[:], gsum[:, 1024:1536], AF.Tanh)
                sgo = work.tile([B, 512], F32, tag="sgo")
                nc.scalar.activation(sgo[:], gsum[:, 1536:2048], AF.Sigmoid)

                t1 = work.tile([B, 512], F32, tag="t1")
                nc.vector.tensor_mul(t1[:], sig_if[:, 0:512], tng[:])
                t2 = work.tile([B, 512], F32, tag="t2")
                nc.vector.tensor_mul(t2[:], sig_if[:, 512:1024], c_cur[:])
                c_new = hc.tile([B, H], F32, tag="c")
                nc.vector.tensor_add(c_new[:], t2[:], t1[:])
                tc2 = work.tile([B, 512], F32, tag="tc2")
                nc.scalar.activation(tc2[:], c_new[:], AF.Tanh)
                h_new = hc.tile([B, H], F32, tag="h")
                nc.vector.tensor_mul(h_new[:], sgo[:], tc2[:])
                c_cur = c_new

                hT = work.tile([128, 4 * B], F32, tag="hT")
                transpose_to(h_new, hT)
                hhiT = work.tile([128, 4 * B], F16, tag="hhiT")
                nc.vector.tensor_copy(hhiT[:], hT[:])
                hres = work.tile([128, 4 * B], F32, tag="hres")
                nc.vector.tensor_sub(hres[:], hT[:], hhiT[:])
                hloT = work.tile([128, 4 * B], F16, tag="hloT")
                nc.vector.tensor_copy(hloT[:], hres[:])

                # --- FC in bank chunks; ACT copy to SBUF; per-chunk top-8 ---
                cands = small.tile([B, NQ * 8], F32, tag="cands")
                cidx = small.tile([B, NQ * 8], U32, tag="cidx")
                logits = lgp.tile([B, VL], F32, tag="logits")
                for q, (off, w) in enumerate(CHUNKS):
                    pf = pfc.tile([B, 512], F32, tag="pfc")
                    for k in range(4):
                        nc.tensor.matmul(
                            pf[:, :w], lhsT=hhiT[:, k * B:(k + 1) * B],
                            rhs=wfchi[k][:, off:off + w],
                            start=(k == 0), stop=False)
                    for k in range(4):
                        nc.tensor.matmul(
                            pf[:, :w], lhsT=hloT[:, k * B:(k + 1) * B],
                            rhs=wfchi[k][:, off:off + w],
                            start=False, stop=False)
                        nc.tensor.matmul(
                            pf[:, :w], lhsT=hhiT[:, k * B:(k + 1) * B],
                            rhs=wfclo[k][:, off:off + w],
                            start=False, stop=False)
                    nc.tensor.matmul(
                        pf[:, :w], lhsT=ones1[:], rhs=bfc[:, off:off + w],
                        start=False, stop=True)
                    lg = logits[:, off:off + w]
                    nc.scalar.copy(lg, pf[:, :w])
                    nc.sync.dma_start(out_d[t][:, off:off + w], lg)
                    if not no_max:
                        nc.vector.max(out=cands[:, q * 8:(q + 1) * 8], in_=lg)
                        nc.vector.max_index(
                            out=cidx[:, q * 8:(q + 1) * 8],
                            in_max=cands[:, q * 8:(q + 1) * 8], in_values=lg)

                # --- combine chunk winners (exact, first-occurrence ties) ---
                pack = small.tile([B, 2], F32, tag="pack")
                if no_max:
                    nc.vector.tensor_scalar(
                        out=pack[:, 0:1], in0=hT[:B, 0:1], scalar1=0.0,
                        scalar2=None, op0=OP.mult)
                    nc.vector.tensor_copy(pack[:, 1:2], pack[:, 0:1])
                else:
                    wv = small.tile([B, 8], F32, tag="wv")
                    nc.vector.max(out=wv[:], in_=cands[:])
                    msk = small.tile([B, NQ * 8], F32, tag="msk")
                    nc.vector.tensor_scalar(
                        out=msk[:], in0=cands[:], scalar1=wv[:, 0:1], scalar2=None,
                        op0=OP.is_equal)
                    idxf = small.tile([B, NQ * 8], F32, tag="idxf")
                    nc.vector.tensor_copy(idxf[:], cidx[:])
                    gidx = small.tile([B, NQ * 8], F32, tag="gidx")
                    nc.vector.tensor_add(gidx[:], idxf[:], qbase[:])
                    gneg = small.tile([B, NQ * 8], F32, tag="gneg")
                    nc.vector.tensor_scalar(
                        out=gneg[:], in0=gidx[:], scalar1=-1.0, scalar2=48000.0,
                        op0=OP.mult, op1=OP.add)
                    gsel = small.tile([B, NQ * 8], F32, tag="gsel")
                    nc.vector.tensor_mul(gsel[:], msk[:], gneg[:])
                    w2 = small.tile([B, 8], F32, tag="w2")
                    nc.vector.max(out=w2[:], in_=gsel[:])
                    # local best value / global id
                    nc.vector.tensor_copy(pack[:, 0:1], wv[:, 0:1])
                    lid = small.tile([B, 1], F32, tag="lid")
                    nc.vector.tensor_scalar(
                        out=lid[:], in0=w2[:, 0:1], scalar1=-1.0, scalar2=48000.0,
                        op0=OP.mult, op1=OP.add)
                    nc.vector.tensor_add(pack[:, 1:2], lid[:], vb64[:])

                # --- exchange + global winner ---
                if no_cc:
                    arr = small.tile([B, 16], F32, tag="arr")
                    for cci in range(8):
                        nc.vector.tensor_copy(
                            arr[:, :].rearrange("b (c j) -> b c j", j=2)[:, cci],
                            pack[:])
                else:
                    cc_in = dram.tile([B, 2], F32, tag="cci")
                    cc_out = dramsh.tile([NCORES, B, 2], F32, addr_space="Shared",
                                         tag="cco")
                    nc.sync.dma_start(cc_in[:], pack[:])
                    nc.gpsimd.collective_compute(
                        "AllGather", OP.bypass,
                        replica_groups=[list(range(NCORES))],
                        ins=[cc_in[:]], outs=[cc_out[:]])
                    arr = small.tile([B, 16], F32, tag="arr")
                    nc.sync.dma_start(
                        arr[:, :].rearrange("b (c j) -> b c j", j=2),
                        cc_out[:].rearrange("c b j -> b c j"))

                vals = arr[:, :].rearrange("b (c j) -> b c j", j=2)[:, :, 0]
                gids = arr[:, :].rearrange("b (c j) -> b c j", j=2)[:, :, 1]
                wmax = small.tile([B, 8], F32, tag="wmax")
                nc.vector.max(out=wmax[:], in_=vals)
                msk2 = small.tile([B, 8], F32, tag="msk2")
                nc.vector.tensor_scalar(
                    out=msk2[:], in0=vals, scalar1=wmax[:, 0:1], scalar2=None,
                    op0=OP.is_equal)
                gneg2 = small.tile([B, 8], F32, tag="gneg2")
                nc.vector.tensor_scalar(
                    out=gneg2[:], in0=gids, scalar1=-1.0, scalar2=40000.0,
                    op0=OP.mult, op1=OP.add)
                gsel2 = small.tile([B, 8], F32, tag="gsel2")
                nc.vector.tensor_mul(gsel2[:], msk2[:], gneg2[:])
                w22 = small.tile([B, 8], F32, tag="w22")
                nc.vector.max(out=w22[:], in_=gsel2[:])
                tokf = small.tile([B, 1], F32, tag="tokf")
                nc.vector.tensor_scalar(
                    out=tokf[:], in0=w22[:, 0:1], scalar1=-1.0, scalar2=40000.0,
                    op0=OP.mult, op1=OP.add)
                tok = small.tile([B, 1], I32, tag="tok")
                nc.vector.tensor_copy(tok[:], tokf[:])

    nc.compile()
    return nc


def _prep_inputs(inputs):
    enc = np.ascontiguousarray(np.asarray(inputs["encoder_outputs"], np.float32))
    captions = np.asarray(inputs["captions"])
    emb = np.asarray(inputs["embedding"], np.float32)
    W_ih = np.asarray(inputs["W_ih"], np.float32)
    b_ih = np.asarray(inputs["b_ih"], np.float32)
    W_hh = np.asarray(inputs["W_hh"], np.float32)
    b_hh = np.asarray(inputs["b_hh"], np.float32)
    W_fc = np.asarray(inputs["W_fc"], np.float32)
    b_fc = np.asarray(inputs["b_fc"], np.float32)
    W_init_h = np.asarray(inputs["W_init_h"], np.float32)
    b_init_h = np.asarray(inputs["b_init_h"], np.float32)
    W_init_c = np.asarray(inputs["W_init_c"], np.float32)
    b_init_c = np.asarray(inputs["b_init_c"], np.float32)

    gtab = (emb @ W_ih.T + (b_ih + b_hh)).astype(np.float32)
    whhT = np.ascontiguousarray(W_hh.T.reshape(4, 128, GD))
    winitT = np.ascontiguousarray(
        (np.concatenate([W_init_h, W_init_c], axis=0) / np.float32(NPIX))
        .T.reshape(4, 128, 1024))
    binit = np.concatenate([b_init_h, b_init_c]).reshape(1, 1024)
    tok0 = np.ascontiguousarray(captions[:, 0].astype(np.int32).reshape(B, 1))

    blk = np.zeros((128, 13 * 8), np.float32)
    for k in range(13):
        for i in range(128):
            r = k * 128 + i
            if r < BL * NPIX:
                blk[i, k * 8 + r // NPIX] = 1.0

    in_maps = []
    for c in range(NCORES):
        enc_c = enc[c * BL:(c + 1) * BL].reshape(BL * NPIX, H)
        enc_pad = np.zeros((13 * 128, H), np.float32)
        enc_pad[:BL * NPIX] = enc_c
        wfc_slice = W_fc[c * VL:(c + 1) * VL]
        wfcT = wfc_slice.T.astype(np.float32)
        wfcT_hi = wfcT.astype(np.float16)
        wfcT_lo = (wfcT - wfcT_hi.astype(np.float32)).astype(np.float16)
        in_maps.append({
            "gtab": gtab,
            "whhT": whhT,
            "wfcThi": np.ascontiguousarray(wfcT_hi.reshape(4, 128, VL)),
            "wfcTlo": np.ascontiguousarray(wfcT_lo.reshape(4, 128, VL)),
            "winitT": winitT,
            "bfc": np.ascontiguousarray(b_fc[c * VL:(c + 1) * VL].reshape(1, VL)),
            "binit": binit,
            "enc": enc_pad.reshape(13, 128, H),
            "blkdiag": blk,
            "tok0": tok0,
            "vbase": np.full((B, 1), c * VL, np.float32),
        })
    return in_maps


def kernel(**inputs) -> np.ndarray:
    if "nc" not in _CACHE:
        _CACHE["nc"] = _build_nc()
    nc = _CACHE["nc"]
    in_maps = _prep_inputs(inputs)
    res = run_bass_kernel_spmd(nc, in_maps, list(range(NCORES)))
    out = np.zeros((B, T, V), np.float32)
    for c in range(NCORES):
        lg = res.results[c]["logits"][:NSTEPS]     # [31, 64, VL]
        out[:, 1:, c * VL:(c + 1) * VL] = lg.transpose(1, 0, 2)
    return out



# revision 3
# speedup vs baseline: 2737.3409x; 2737.3409x over previous
"""Trainium2 Bass kernel for nn_DecoderLSTM_noAttention — collective-free.

Strategy: every core runs the FULL greedy decode independently (replicated
compute, zero cross-core communication — collectives cost ~1 network RTT each
under the axon tunnel and dominated the old kernel's runtime).

Per step:
- indirect-gather embedding rows for the current tokens [64, 512]
- transpose x on PE; gates = x@W_ih.T + h@W_hh.T + (b_ih+b_hh) in fp32 on PE
- pointwise LSTM via activation tables (same numerics as before)
- FC over the FULL vocab with fp16 hi/lo 3-pass matmuls (error ~1e-7,
  preserves exact argmax vs the fp32 reference); W_fc streamed from DRAM
  (66 MB/step) since the full-vocab weights don't fit in SBUF
- per-chunk DVE max/max_index straight from PSUM; exact first-occurrence
  tie-break combine; next token fully local
- logits written out as fp16 (output tolerance is 2e-2; fp16 adds ~5e-4)

Output: each core writes identical [31, 64, 32000] fp16 logits; the host uses
core 0's copy.
"""
import numpy as np

import concourse.bass as bass
import concourse.bacc as bacc
import concourse.tile as tile
from concourse import mybir
from concourse.bass_utils import run_bass_kernel_spmd
from concourse.masks import make_identity

F32 = mybir.dt.float32
F16 = mybir.dt.float16
I32 = mybir.dt.int32
U32 = mybir.dt.uint32
AF = mybir.ActivationFunctionType
OP = mybir.AluOpType

B = 64
H = 512
V = 32000
T = 32
NPIX = 196
NCORES = 8
NSTEPS = T - 1
GD = 2048

# FC chunking over the full vocab: 62x512 + 1x256
CHUNKS = [(q * 512, 512) for q in range(62)] + [(31744, 256)]
NQ = len(CHUNKS)
CPAD = 512  # per-chunk padded width in the streamed weight layout

_CACHE = {}


def _build_nc(nsteps=NSTEPS, out_slots=NSTEPS):
    nc = bacc.Bacc("TRN2", target_bir_lowering=False, debug=False,
                   num_devices=NCORES)

    emb_d = nc.dram_tensor("emb", [V, H], F32, kind="ExternalInput")
    wih_d = nc.dram_tensor("wihT", [4, 128, GD], F32, kind="ExternalInput")
    whh_d = nc.dram_tensor("whhT", [4, 128, GD], F32, kind="ExternalInput")
    # streamed FC weights: [chunk][128][k*1024 + (hi=0/lo=512) + col];
    # cols 4096:4608 row 0 carry the bias slice for this chunk
    wfc_d = nc.dram_tensor("wfcs", [NQ, 128, 4 * 2 * CPAD + CPAD], F16,
                           kind="ExternalInput")
    winit_d = nc.dram_tensor("winitT", [4, 128, 1024], F32, kind="ExternalInput")
    bg_d = nc.dram_tensor("bgate", [1, GD], F32, kind="ExternalInput")
    binit_d = nc.dram_tensor("binit", [1, 1024], F32, kind="ExternalInput")
    summ_d = nc.dram_tensor("summary", [B, H], F32, kind="ExternalInput")
    tok0_d = nc.dram_tensor("tok0", [B, 1], I32, kind="ExternalInput")
    qbase_d = nc.dram_tensor("qbase", [B, NQ * 8], F32, kind="ExternalInput")

    out_d = nc.dram_tensor("logits", [out_slots, B, V], F16,
                           kind="ExternalOutput")

    with tile.TileContext(nc) as tc:
        import contextlib
        with contextlib.ExitStack() as ctx:
            const = ctx.enter_context(tc.tile_pool(name="const", bufs=1))
            work = ctx.enter_context(tc.tile_pool(name="work", bufs=1))
            hc = ctx.enter_context(tc.tile_pool(name="hc", bufs=2))
            small = ctx.enter_context(tc.tile_pool(name="small", bufs=2))
            stream = ctx.enter_context(tc.tile_pool(name="stream", bufs=4))
            lgout = ctx.enter_context(tc.tile_pool(name="lgout", bufs=3))
            ptr = ctx.enter_context(tc.tile_pool(name="ptr", bufs=1, space="PSUM"))
            pg = ctx.enter_context(tc.tile_pool(name="pg", bufs=1, space="PSUM"))
            pfc = ctx.enter_context(tc.tile_pool(name="pfc", bufs=3, space="PSUM"))

            # ---- constants / resident weights ----
            ident = const.tile([B, B], F32)
            make_identity(nc, ident[:])
            ones16 = const.tile([1, B], F16)
            nc.vector.memset(ones16[:], 1.0)
            qbase = const.tile([B, NQ * 8], F32)
            nc.sync.dma_start(qbase[:], qbase_d[:])

            wih = []
            whh = []
            for k in range(4):
                w = const.tile([128, GD], F32, tag=f"wih{k}")
                nc.sync.dma_start(w[:], wih_d[k])
                wih.append(w)
                w = const.tile([128, GD], F32, tag=f"whh{k}")
                nc.sync.dma_start(w[:], whh_d[k])
                whh.append(w)
            bg = const.tile([1, GD], F32)
            nc.sync.dma_start(bg[:], bg_d[:])
            binit = const.tile([1, 1024], F32)
            nc.sync.dma_start(binit[:], binit_d[:])
            onesf = const.tile([1, B], F32)
            nc.vector.memset(onesf[:], 1.0)

            def transpose_to(src, dst_tile):
                """src SBUF [B, 512] f32 -> dst SBUF [128, 4*B] (k-packed)."""
                for k in range(4):
                    pt = ptr.tile([128, B], F32, tag="ptr")
                    nc.tensor.transpose(
                        out=pt[:], in_=src[:, k * 128:(k + 1) * 128],
                        identity=ident[:])
                    nc.scalar.copy(dst_tile[:, k * B:(k + 1) * B], pt[:])

            # ---- phase 0: h0/c0 from host-computed encoder mean ----
            summ = work.tile([B, H], F32, tag="gx")
            nc.sync.dma_start(summ[:], summ_d[:])
            sumT = work.tile([128, 4 * B], F32, tag="xT")
            transpose_to(summ, sumT)
            for n in range(2):
                ph = pfc.tile([B, 512], F32, tag="pfc")
                for k in range(4):
                    wi = work.tile([128, 1024], F32, tag="winit")
                    nc.sync.dma_start(wi[:], winit_d[k])
                    nc.tensor.matmul(
                        ph[:], lhsT=sumT[:, k * B:(k + 1) * B],
                        rhs=wi[:, n * 512:(n + 1) * 512],
                        start=(k == 0), stop=False)
                nc.tensor.matmul(
                    ph[:], lhsT=onesf[:], rhs=binit[:, n * 512:(n + 1) * 512],
                    start=False, stop=True)
                dst = hc.tile([B, H], F32, tag=("h" if n == 0 else "c"))
                nc.scalar.copy(dst[:], ph[:])
                if n == 0:
                    h_cur = dst
                else:
                    c_cur = dst

            hT = work.tile([128, 4 * B], F32, tag="hT")
            transpose_to(h_cur, hT)
            hhiT = work.tile([128, 4 * B], F16, tag="hhiT")
            nc.vector.tensor_copy(hhiT[:], hT[:])
            hres = work.tile([128, 4 * B], F32, tag="hres")
            nc.vector.tensor_sub(hres[:], hT[:], hhiT[:])
            hloT = work.tile([128, 4 * B], F16, tag="hloT")
            nc.vector.tensor_copy(hloT[:], hres[:])

            tok = small.tile([B, 1], I32, tag="tok")
            nc.sync.dma_start(tok[:], tok0_d[:])

            # ---- decode steps ----
            for t in range(nsteps):
                # x = emb[tok]
                gx = work.tile([B, H], F32, tag="gx")
                nc.gpsimd.indirect_dma_start(
                    out=gx[:], out_offset=None, in_=emb_d[:],
                    in_offset=bass.IndirectOffsetOnAxis(ap=tok[:, :1], axis=0))
                xT = work.tile([128, 4 * B], F32, tag="xT")
                transpose_to(gx, xT)

                # gates = x@W_ih.T + h@W_hh.T + bg   (fp32 on PE)
                pgt = pg.tile([B, GD], F32, tag="pg")
                for n in range(4):
                    sl = slice(n * 512, (n + 1) * 512)
                    for k in range(4):
                        nc.tensor.matmul(
                            pgt[:, sl], lhsT=xT[:, k * B:(k + 1) * B],
                            rhs=wih[k][:, sl], start=(k == 0), stop=False)
                    for k in range(4):
                        nc.tensor.matmul(
                            pgt[:, sl], lhsT=hT[:, k * B:(k + 1) * B],
                            rhs=whh[k][:, sl], start=False, stop=False)
                    nc.tensor.matmul(
                        pgt[:, sl], lhsT=onesf[:], rhs=bg[:, sl],
                        start=False, stop=True)

                # pointwise LSTM
                sig_if = work.tile([B, 1024], F32, tag="sigif")
                nc.scalar.activation(sig_if[:], pgt[:, 0:1024], AF.Sigmoid)
                tng = work.tile([B, 512], F32, tag="tng")
                nc.scalar.activation(tng[:], pgt[:, 1024:1536], AF.Tanh)
                sgo = work.tile([B, 512], F32, tag="sgo")
                nc.scalar.activation(sgo[:], pgt[:, 1536:2048], AF.Sigmoid)

                t1 = work.tile([B, 512], F32, tag="t1")
                nc.vector.tensor_mul(t1[:], sig_if[:, 0:512], tng[:])
                t2 = work.tile([B, 512], F32, tag="t2")
                nc.vector.tensor_mul(t2[:], sig_if[:, 512:1024], c_cur[:])
                c_new = hc.tile([B, H], F32, tag="c")
                nc.vector.tensor_add(c_new[:], t2[:], t1[:])
                tc2 = work.tile([B, 512], F32, tag="tc2")
                nc.scalar.activation(tc2[:], c_new[:], AF.Tanh)
                h_new = hc.tile([B, H], F32, tag="h")
                nc.vector.tensor_mul(h_new[:], sgo[:], tc2[:])
                c_cur = c_new

                hT = work.tile([128, 4 * B], F32, tag="hT")
                transpose_to(h_new, hT)
                hhiT = work.tile([128, 4 * B], F16, tag="hhiT")
                nc.vector.tensor_copy(hhiT[:], hT[:])
                hres = work.tile([128, 4 * B], F32, tag="hres")
                nc.vector.tensor_sub(hres[:], hT[:], hhiT[:])
                hloT = work.tile([128, 4 * B], F16, tag="hloT")
                nc.vector.tensor_copy(hloT[:], hres[:])

                # FC over full vocab, streamed fp16 hi/lo 3-pass
                cands = small.tile([B, NQ * 8], F32, tag="cands")
                cidx = small.tile([B, NQ * 8], U32, tag="cidx")
                for q, (off, w) in enumerate(CHUNKS):
                    wt = stream.tile([128, 4 * 2 * CPAD + CPAD], F16, tag="wt")
                    nc.sync.dma_start(wt[:], wfc_d[q])
                    pf = pfc.tile([B, 512], F32, tag="pfc")
                    for k in range(4):
                        nc.tensor.matmul(
                            pf[:, :w], lhsT=hhiT[:, k * B:(k + 1) * B],
                            rhs=wt[:, k * 1024:k * 1024 + w],
                            start=(k == 0), stop=False)
                    for k in range(4):
                        nc.tensor.matmul(
                            pf[:, :w], lhsT=hloT[:, k * B:(k + 1) * B],
                            rhs=wt[:, k * 1024:k * 1024 + w],
                            start=False, stop=False)
                        nc.tensor.matmul(
                            pf[:, :w], lhsT=hhiT[:, k * B:(k + 1) * B],
                            rhs=wt[:, k * 1024 + CPAD:k * 1024 + CPAD + w],
                            start=False, stop=False)
                    nc.tensor.matmul(
                        pf[:, :w], lhsT=ones16[:],
                        rhs=wt[0:1, 4096:4096 + w],
                        start=False, stop=True)
                    nc.vector.max(out=cands[:, q * 8:(q + 1) * 8], in_=pf[:, :w])
                    nc.vector.max_index(
                        out=cidx[:, q * 8:(q + 1) * 8],
                        in_max=cands[:, q * 8:(q + 1) * 8], in_values=pf[:, :w])
                    lg = lgout.tile([B, 512], F16, tag="lg")
                    nc.scalar.copy(lg[:, :w], pf[:, :w])
                    nc.sync.dma_start(out_d[t][:, off:off + w], lg[:, :w])

                # exact global argmax (first-occurrence tie-breaks)
                wv = small.tile([B, 8], F32, tag="wv")
                nc.vector.max(out=wv[:], in_=cands[:])
                msk = small.tile([B, NQ * 8], F32, tag="msk")
                nc.vector.tensor_scalar(
                    out=msk[:], in0=cands[:], scalar1=wv[:, 0:1], scalar2=None,
                    op0=OP.is_equal)
                idxf = small.tile([B, NQ * 8], F32, tag="idxf")
                nc.vector.tensor_copy(idxf[:], cidx[:])
                gidx = small.tile([B, NQ * 8], F32, tag="gidx")
                nc.vector.tensor_add(gidx[:], idxf[:], qbase[:])
                gneg = small.tile([B, NQ * 8], F32, tag="gneg")
                nc.vector.tensor_scalar(
                    out=gneg[:], in0=gidx[:], scalar1=-1.0, scalar2=48000.0,
                    op0=OP.mult, op1=OP.add)
                gsel = small.tile([B, NQ * 8], F32, tag="gsel")
                nc.vector.tensor_mul(gsel[:], msk[:], gneg[:])
                w2 = small.tile([B, 8], F32, tag="w2")
                nc.vector.max(out=w2[:], in_=gsel[:])
                tokf = small.tile([B, 1], F32, tag="tokf")
                nc.vector.tensor_scalar(
                    out=tokf[:], in0=w2[:, 0:1], scalar1=-1.0, scalar2=48000.0,
                    op0=OP.mult, op1=OP.add)
                tok = small.tile([B, 1], I32, tag="tok")
                nc.vector.tensor_copy(tok[:], tokf[:])

    nc.compile()
    return nc


def _prep_inputs(inputs):
    enc = np.asarray(inputs["encoder_outputs"], np.float32)
    captions = np.asarray(inputs["captions"])
    emb = np.ascontiguousarray(np.asarray(inputs["embedding"], np.float32))
    W_ih = np.asarray(inputs["W_ih"], np.float32)
    b_ih = np.asarray(inputs["b_ih"], np.float32)
    W_hh = np.asarray(inputs["W_hh"], np.float32)
    b_hh = np.asarray(inputs["b_hh"], np.float32)
    W_fc = np.asarray(inputs["W_fc"], np.float32)
    b_fc = np.asarray(inputs["b_fc"], np.float32)
    W_init_h = np.asarray(inputs["W_init_h"], np.float32)
    b_init_h = np.asarray(inputs["b_init_h"], np.float32)
    W_init_c = np.asarray(inputs["W_init_c"], np.float32)
    b_init_c = np.asarray(inputs["b_init_c"], np.float32)

    wihT = np.ascontiguousarray(W_ih.T.reshape(4, 128, GD))
    whhT = np.ascontiguousarray(W_hh.T.reshape(4, 128, GD))
    winitT = np.ascontiguousarray(
        np.concatenate([W_init_h, W_init_c], axis=0).T.reshape(4, 128, 1024))
    binit = np.concatenate([b_init_h, b_init_c]).reshape(1, 1024)
    bgate = (b_ih + b_hh).reshape(1, GD)
    summary = enc.mean(axis=1).astype(np.float32)
    tok0 = np.ascontiguousarray(captions[:, 0].astype(np.int32).reshape(B, 1))

    WT = W_fc.T.astype(np.float32)          # [512, 32000]
    Whi = WT.astype(np.float16)
    Wlo = (WT - Whi.astype(np.float32)).astype(np.float16)
    bfc16 = b_fc.astype(np.float16)
    wfcs = np.zeros((NQ, 128, 4 * 2 * CPAD + CPAD), np.float16)
    for q, (off, w) in enumerate(CHUNKS):
        for k in range(4):
            wfcs[q, :, k * 1024:k * 1024 + w] = \
                Whi[k * 128:(k + 1) * 128, off:off + w]
            wfcs[q, :, k * 1024 + CPAD:k * 1024 + CPAD + w] = \
                Wlo[k * 128:(k + 1) * 128, off:off + w]
        wfcs[q, 0, 4096:4096 + w] = bfc16[off:off + w]

    qbase = np.zeros((B, NQ * 8), np.float32)
    for q, (off, _w) in enumerate(CHUNKS):
        qbase[:, q * 8:(q + 1) * 8] = float(off)

    in_map = {
        "emb": emb,
        "wihT": wihT,
        "whhT": whhT,
        "wfcs": wfcs,
        "winitT": winitT,
        "bgate": bgate,
        "binit": binit,
        "summary": summary,
        "tok0": tok0,
        "qbase": qbase,
    }
    return [in_map for _ in range(NCORES)]


def kernel(**inputs) -> np.ndarray:
    if "nc" not in _CACHE:
        _CACHE["nc"] = _build_nc()
    nc = _CACHE["nc"]
    in_maps = _prep_inputs(inputs)
    res = run_bass_kernel_spmd(nc, in_maps, list(range(NCORES)))
    out = np.zeros((B, T, V), np.float32)
    lg = res.results[0]["logits"][:NSTEPS].astype(np.float32)  # [31, 64, V]
    out[:, 1:, :] = lg.transpose(1, 0, 2)
    return out


# revision 4
# speedup vs baseline: 2905.1908x; 1.0613x over previous
"""Trainium2 Bass kernel for nn_DecoderLSTM_noAttention — collective-free.

Strategy: every core runs the FULL greedy decode independently (replicated
compute, zero cross-core communication — collectives cost ~1 network RTT each
under the axon tunnel and dominated the old kernel's runtime).

Per step:
- indirect-gather embedding rows for the current tokens [64, 512]
- transpose x on PE; gates = x@W_ih.T + h@W_hh.T + (b_ih+b_hh) in fp32 on PE
- pointwise LSTM via activation tables (same numerics as before)
- FC over the FULL vocab with fp16 hi/lo 3-pass matmuls (error ~1e-7,
  preserves exact argmax vs the fp32 reference); W_fc streamed from DRAM
  (66 MB/step) since the full-vocab weights don't fit in SBUF
- per-chunk DVE max/max_index straight from PSUM; exact first-occurrence
  tie-break combine; next token fully local
- logits written out as fp16 (output tolerance is 2e-2; fp16 adds ~5e-4)

Output: each core writes identical [31, 64, 32000] fp16 logits; the host uses
core 0's copy.
"""
import numpy as np

import concourse.bass as bass
import concourse.bacc as bacc
import concourse.tile as tile
from concourse import mybir
from concourse.bass_utils import run_bass_kernel_spmd
from concourse.masks import make_identity

F32 = mybir.dt.float32
F16 = mybir.dt.float16
I32 = mybir.dt.int32
U32 = mybir.dt.uint32
AF = mybir.ActivationFunctionType
OP = mybir.AluOpType

B = 64
H = 512
V = 32000
T = 32
NPIX = 196
NCORES = 8
NSTEPS = T - 1
GD = 2048

# FC chunking over the full vocab: 62x512 + 1x256
CHUNKS = [(q * 512, 512) for q in range(62)] + [(31744, 256)]
NQ = len(CHUNKS)
CPAD = 512  # per-chunk padded width in the streamed weight layout

_CACHE = {}


def _build_nc(nsteps=NSTEPS, out_slots=NSTEPS):
    nc = bacc.Bacc("TRN2", target_bir_lowering=False, debug=False,
                   num_devices=NCORES)

    emb_d = nc.dram_tensor("emb", [V, H], F32, kind="ExternalInput")
    wih_d = nc.dram_tensor("wihT", [4, 128, GD], F32, kind="ExternalInput")
    whh_d = nc.dram_tensor("whhT", [4, 128, GD], F32, kind="ExternalInput")
    # streamed FC weights: [chunk][128][k*1024 + (hi=0/lo=512) + col]
    wfc_d = nc.dram_tensor("wfcs", [NQ, 128, 4 * 2 * CPAD], F16,
                           kind="ExternalInput")
    winit_d = nc.dram_tensor("winitT", [4, 128, 1024], F32, kind="ExternalInput")
    bg_d = nc.dram_tensor("bgate", [1, GD], F32, kind="ExternalInput")
    binit_d = nc.dram_tensor("binit", [1, 1024], F32, kind="ExternalInput")
    summ_d = nc.dram_tensor("summary", [B, H], F32, kind="ExternalInput")
    tok0_d = nc.dram_tensor("tok0", [B, 1], I32, kind="ExternalInput")
    qbase_d = nc.dram_tensor("qbase", [B, NQ * 8], F32, kind="ExternalInput")
    bsel_d = nc.dram_tensor("bsel", [64, NQ * B], F16, kind="ExternalInput")
    bpack_d = nc.dram_tensor("bpack", [64, 512], F16, kind="ExternalInput")

    out_d = nc.dram_tensor("logits", [out_slots, B, V], F16,
                           kind="ExternalOutput")

    with tile.TileContext(nc) as tc:
        import contextlib
        with contextlib.ExitStack() as ctx:
            const = ctx.enter_context(tc.tile_pool(name="const", bufs=1))
            work = ctx.enter_context(tc.tile_pool(name="work", bufs=1))
            hc = ctx.enter_context(tc.tile_pool(name="hc", bufs=2))
            small = ctx.enter_context(tc.tile_pool(name="small", bufs=2))
            stream = ctx.enter_context(tc.tile_pool(name="stream", bufs=5))
            lgout = ctx.enter_context(tc.tile_pool(name="lgout", bufs=3))
            ptr = ctx.enter_context(tc.tile_pool(name="ptr", bufs=1, space="PSUM"))
            pg = ctx.enter_context(tc.tile_pool(name="pg", bufs=1, space="PSUM"))
            pfc = ctx.enter_context(tc.tile_pool(name="pfc", bufs=3, space="PSUM"))

            # ---- constants / resident weights ----
            ident = const.tile([B, B], F32)
            make_identity(nc, ident[:])
            ones16 = const.tile([1, B], F16)
            nc.vector.memset(ones16[:], 1.0)
            qbase = const.tile([B, NQ * 8], F32)
            nc.sync.dma_start(qbase[:], qbase_d[:])

            wih = []
            whh = []
            for k in range(4):
                w = const.tile([128, GD], F32, tag=f"wih{k}")
                nc.sync.dma_start(w[:], wih_d[k])
                wih.append(w)
                w = const.tile([128, GD], F32, tag=f"whh{k}")
                nc.sync.dma_start(w[:], whh_d[k])
                whh.append(w)
            bg = const.tile([1, GD], F32)
            nc.sync.dma_start(bg[:], bg_d[:])
            binit = const.tile([1, 1024], F32)
            nc.sync.dma_start(binit[:], binit_d[:])
            onesf = const.tile([1, B], F32)
            nc.vector.memset(onesf[:], 1.0)
            bsel = const.tile([64, NQ * B], F16)
            nc.sync.dma_start(bsel[:], bsel_d[:])
            bpack = const.tile([64, 512], F16)
            nc.sync.dma_start(bpack[:], bpack_d[:])

            def transpose_to(src, dst_tile):
                """src SBUF [B, 512] f32 -> dst SBUF [128, 4*B] (k-packed)."""
                for k in range(4):
                    pt = ptr.tile([128, B], F32, tag="ptr")
                    nc.tensor.transpose(
                        out=pt[:], in_=src[:, k * 128:(k + 1) * 128],
                        identity=ident[:])
                    nc.scalar.copy(dst_tile[:, k * B:(k + 1) * B], pt[:])

            # ---- phase 0: h0/c0 from host-computed encoder mean ----
            summ = work.tile([B, H], F32, tag="gx")
            nc.sync.dma_start(summ[:], summ_d[:])
            sumT = work.tile([128, 4 * B], F32, tag="xT")
            transpose_to(summ, sumT)
            for n in range(2):
                ph = pfc.tile([B, 512], F32, tag="pfc")
                for k in range(4):
                    wi = work.tile([128, 1024], F32, tag="winit")
                    nc.sync.dma_start(wi[:], winit_d[k])
                    nc.tensor.matmul(
                        ph[:], lhsT=sumT[:, k * B:(k + 1) * B],
                        rhs=wi[:, n * 512:(n + 1) * 512],
                        start=(k == 0), stop=False)
                nc.tensor.matmul(
                    ph[:], lhsT=onesf[:], rhs=binit[:, n * 512:(n + 1) * 512],
                    start=False, stop=True)
                dst = hc.tile([B, H], F32, tag=("h" if n == 0 else "c"))
                nc.scalar.copy(dst[:], ph[:])
                if n == 0:
                    h_cur = dst
                else:
                    c_cur = dst

            hT = work.tile([128, 4 * B], F32, tag="hT")
            transpose_to(h_cur, hT)
            hhiT = work.tile([128, 4 * B], F16, tag="hhiT")
            nc.vector.tensor_copy(hhiT[:], hT[:])
            hres = work.tile([128, 4 * B], F32, tag="hres")
            nc.vector.tensor_sub(hres[:], hT[:], hhiT[:])
            hloT = work.tile([128, 4 * B], F16, tag="hloT")
            nc.vector.tensor_copy(hloT[:], hres[:])

            tok = small.tile([B, 1], I32, tag="tok")
            nc.sync.dma_start(tok[:], tok0_d[:])

            # ---- decode steps ----
            for t in range(nsteps):
                # x = emb[tok]
                gx = work.tile([B, H], F32, tag="gx")
                nc.gpsimd.indirect_dma_start(
                    out=gx[:], out_offset=None, in_=emb_d[:],
                    in_offset=bass.IndirectOffsetOnAxis(ap=tok[:, :1], axis=0))
                xT = work.tile([128, 4 * B], F32, tag="xT")
                transpose_to(gx, xT)

                # gates = x@W_ih.T + h@W_hh.T + bg   (fp32 on PE)
                pgt = pg.tile([B, GD], F32, tag="pg")
                for n in range(4):
                    sl = slice(n * 512, (n + 1) * 512)
                    for k in range(4):
                        nc.tensor.matmul(
                            pgt[:, sl], lhsT=xT[:, k * B:(k + 1) * B],
                            rhs=wih[k][:, sl], start=(k == 0), stop=False)
                    for k in range(4):
                        nc.tensor.matmul(
                            pgt[:, sl], lhsT=hT[:, k * B:(k + 1) * B],
                            rhs=whh[k][:, sl], start=False, stop=False)
                    nc.tensor.matmul(
                        pgt[:, sl], lhsT=onesf[:], rhs=bg[:, sl],
                        start=False, stop=True)

                # pointwise LSTM
                sig_if = work.tile([B, 1024], F32, tag="sigif")
                nc.scalar.activation(sig_if[:], pgt[:, 0:1024], AF.Sigmoid)
                tng = work.tile([B, 512], F32, tag="tng")
                nc.scalar.activation(tng[:], pgt[:, 1024:1536], AF.Tanh)
                sgo = work.tile([B, 512], F32, tag="sgo")
                nc.scalar.activation(sgo[:], pgt[:, 1536:2048], AF.Sigmoid)

                t1 = work.tile([B, 512], F32, tag="t1")
                nc.vector.tensor_mul(t1[:], sig_if[:, 0:512], tng[:])
                t2 = work.tile([B, 512], F32, tag="t2")
                nc.vector.tensor_mul(t2[:], sig_if[:, 512:1024], c_cur[:])
                c_new = hc.tile([B, H], F32, tag="c")
                nc.vector.tensor_add(c_new[:], t2[:], t1[:])
                tc2 = work.tile([B, 512], F32, tag="tc2")
                nc.scalar.activation(tc2[:], c_new[:], AF.Tanh)
                h_new = hc.tile([B, H], F32, tag="h")
                nc.vector.tensor_mul(h_new[:], sgo[:], tc2[:])
                c_cur = c_new

                hT = work.tile([128, 4 * B], F32, tag="hT")
                transpose_to(h_new, hT)
                hhiT = work.tile([128, 4 * B], F16, tag="hhiT")
                nc.vector.tensor_copy(hhiT[:], hT[:])
                hres = work.tile([128, 4 * B], F32, tag="hres")
                nc.vector.tensor_sub(hres[:], hT[:], hhiT[:])
                hloT = work.tile([128, 4 * B], F16, tag="hloT")
                nc.vector.tensor_copy(hloT[:], hres[:])

                # FC over full vocab, streamed fp16 hi/lo 3-pass
                cands = small.tile([B, NQ * 8], F32, tag="cands")
                cidx = small.tile([B, NQ * 8], U32, tag="cidx")
                for q, (off, w) in enumerate(CHUNKS):
                    wt = stream.tile([128, 4 * 2 * CPAD], F16, tag="wt")
                    nc.sync.dma_start(wt[:], wfc_d[q])
                    pf = pfc.tile([B, 512], F32, tag="pfc")
                    for k in range(4):
                        nc.tensor.matmul(
                            pf[:, :w], lhsT=hhiT[:, k * B:(k + 1) * B],
                            rhs=wt[:, k * 1024:k * 1024 + w],
                            start=(k == 0), stop=False)
                    for k in range(4):
                        nc.tensor.matmul(
                            pf[:, :w], lhsT=hloT[:, k * B:(k + 1) * B],
                            rhs=wt[:, k * 1024:k * 1024 + w],
                            start=False, stop=False)
                        nc.tensor.matmul(
                            pf[:, :w], lhsT=hhiT[:, k * B:(k + 1) * B],
                            rhs=wt[:, k * 1024 + CPAD:k * 1024 + CPAD + w],
                            start=False, stop=False)
                    nc.tensor.matmul(
                        pf[:, :w], lhsT=bsel[:, q * B:(q + 1) * B],
                        rhs=bpack[:, :w], start=False, stop=True)
                    nc.vector.max(out=cands[:, q * 8:(q + 1) * 8], in_=pf[:, :w])
                    nc.vector.max_index(
                        out=cidx[:, q * 8:(q + 1) * 8],
                        in_max=cands[:, q * 8:(q + 1) * 8], in_values=pf[:, :w])
                    lg = lgout.tile([B, 512], F16, tag="lg")
                    nc.scalar.copy(lg[:, :w], pf[:, :w])
                    nc.sync.dma_start(out_d[t][:, off:off + w], lg[:, :w])

                # exact global argmax (first-occurrence tie-breaks)
                wv = small.tile([B, 8], F32, tag="wv")
                nc.vector.max(out=wv[:], in_=cands[:])
                msk = small.tile([B, NQ * 8], F32, tag="msk")
                nc.vector.tensor_scalar(
                    out=msk[:], in0=cands[:], scalar1=wv[:, 0:1], scalar2=None,
                    op0=OP.is_equal)
                idxf = small.tile([B, NQ * 8], F32, tag="idxf")
                nc.vector.tensor_copy(idxf[:], cidx[:])
                gidx = small.tile([B, NQ * 8], F32, tag="gidx")
                nc.vector.tensor_add(gidx[:], idxf[:], qbase[:])
                gneg = small.tile([B, NQ * 8], F32, tag="gneg")
                nc.vector.tensor_scalar(
                    out=gneg[:], in0=gidx[:], scalar1=-1.0, scalar2=48000.0,
                    op0=OP.mult, op1=OP.add)
                gsel = small.tile([B, NQ * 8], F32, tag="gsel")
                nc.vector.tensor_mul(gsel[:], msk[:], gneg[:])
                w2 = small.tile([B, 8], F32, tag="w2")
                nc.vector.max(out=w2[:], in_=gsel[:])
                tokf = small.tile([B, 1], F32, tag="tokf")
                nc.vector.tensor_scalar(
                    out=tokf[:], in0=w2[:, 0:1], scalar1=-1.0, scalar2=48000.0,
                    op0=OP.mult, op1=OP.add)
                tok = small.tile([B, 1], I32, tag="tok")
                nc.vector.tensor_copy(tok[:], tokf[:])

    nc.compile()
    return nc


def _prep_inputs(inputs):
    enc = np.asarray(inputs["encoder_outputs"], np.float32)
    captions = np.asarray(inputs["captions"])
    emb = np.ascontiguousarray(np.asarray(inputs["embedding"], np.float32))
    W_ih = np.asarray(inputs["W_ih"], np.float32)
    b_ih = np.asarray(inputs["b_ih"], np.float32)
    W_hh = np.asarray(inputs["W_hh"], np.float32)
    b_hh = np.asarray(inputs["b_hh"], np.float32)
    W_fc = np.asarray(inputs["W_fc"], np.float32)
    b_fc = np.asarray(inputs["b_fc"], np.float32)
    W_init_h = np.asarray(inputs["W_init_h"], np.float32)
    b_init_h = np.asarray(inputs["b_init_h"], np.float32)
    W_init_c = np.asarray(inputs["W_init_c"], np.float32)
    b_init_c = np.asarray(inputs["b_init_c"], np.float32)

    wihT = np.ascontiguousarray(W_ih.T.reshape(4, 128, GD))
    whhT = np.ascontiguousarray(W_hh.T.reshape(4, 128, GD))
    winitT = np.ascontiguousarray(
        np.concatenate([W_init_h, W_init_c], axis=0).T.reshape(4, 128, 1024))
    binit = np.concatenate([b_init_h, b_init_c]).reshape(1, 1024)
    bgate = (b_ih + b_hh).reshape(1, GD)
    summary = enc.mean(axis=1).astype(np.float32)
    tok0 = np.ascontiguousarray(captions[:, 0].astype(np.int32).reshape(B, 1))

    WT = W_fc.T.astype(np.float32)          # [512, 32000]
    Whi = WT.astype(np.float16)
    Wlo = (WT - Whi.astype(np.float32)).astype(np.float16)
    bfc16 = b_fc.astype(np.float16)
    wfcs = np.zeros((NQ, 128, 4 * 2 * CPAD), np.float16)
    for q, (off, w) in enumerate(CHUNKS):
        for k in range(4):
            wfcs[q, :, k * 1024:k * 1024 + w] = \
                Whi[k * 128:(k + 1) * 128, off:off + w]
            wfcs[q, :, k * 1024 + CPAD:k * 1024 + CPAD + w] = \
                Wlo[k * 128:(k + 1) * 128, off:off + w]
    bsel = np.zeros((64, NQ * B), np.float16)
    for q in range(NQ):
        bsel[q, q * B:(q + 1) * B] = 1.0
    bpack = np.zeros((64, 512), np.float16)
    for q, (off, w) in enumerate(CHUNKS):
        bpack[q, :w] = bfc16[off:off + w]

    qbase = np.zeros((B, NQ * 8), np.float32)
    for q, (off, _w) in enumerate(CHUNKS):
        qbase[:, q * 8:(q + 1) * 8] = float(off)

    in_map = {
        "emb": emb,
        "wihT": wihT,
        "whhT": whhT,
        "wfcs": wfcs,
        "winitT": winitT,
        "bgate": bgate,
        "binit": binit,
        "summary": summary,
        "tok0": tok0,
        "qbase": qbase,
        "bsel": bsel,
        "bpack": bpack,
    }
    return [in_map for _ in range(NCORES)]


def kernel(**inputs) -> np.ndarray:
    if "nc" not in _CACHE:
        _CACHE["nc"] = _build_nc()
    nc = _CACHE["nc"]
    in_maps = _prep_inputs(inputs)
    res = run_bass_kernel_spmd(nc, in_maps, list(range(NCORES)))
    out = np.zeros((B, T, V), np.float32)
    lg = res.results[0]["logits"][:NSTEPS].astype(np.float32)  # [31, 64, V]
    out[:, 1:, :] = lg.transpose(1, 0, 2)
    return out


# revision 5
# speedup vs baseline: 3058.6238x; 1.0528x over previous
"""Trainium2 Bass kernel for nn_DecoderLSTM_noAttention — collective-free.

Strategy: every core runs the FULL greedy decode independently (replicated
compute, zero cross-core communication — collectives cost ~1 network RTT each
under the axon tunnel and dominated the old kernel's runtime).

Per step:
- indirect-gather embedding rows for the current tokens [64, 512]
- transpose x on PE; gates = x@W_ih.T + h@W_hh.T + (b_ih+b_hh) in fp32 on PE
- pointwise LSTM via activation tables (same numerics as before)
- FC over the FULL vocab with fp16 hi/lo 3-pass matmuls (error ~1e-7,
  preserves exact argmax vs the fp32 reference); W_fc streamed from DRAM
  (66 MB/step) since the full-vocab weights don't fit in SBUF
- per-chunk DVE max/max_index straight from PSUM; exact first-occurrence
  tie-break combine; next token fully local
- logits written out as fp16 (output tolerance is 2e-2; fp16 adds ~5e-4)

Output: each core writes identical [31, 64, 32000] fp16 logits; the host uses
core 0's copy.
"""
import numpy as np

import concourse.bass as bass
import concourse.bacc as bacc
import concourse.tile as tile
from concourse import mybir
from concourse.bass_utils import run_bass_kernel_spmd
from concourse.masks import make_identity

F32 = mybir.dt.float32
F16 = mybir.dt.float16
I32 = mybir.dt.int32
U32 = mybir.dt.uint32
AF = mybir.ActivationFunctionType
OP = mybir.AluOpType

B = 64
H = 512
V = 32000
T = 32
NPIX = 196
NCORES = 8
NSTEPS = T - 1
GD = 2048

# FC chunking over the full vocab: 62x512 + 1x256
CHUNKS = [(q * 512, 512) for q in range(62)] + [(31744, 256)]
NQ = len(CHUNKS)
CPAD = 512  # per-chunk padded width in the streamed weight layout

_CACHE = {}


def _build_nc(nsteps=NSTEPS, out_slots=NSTEPS):
    nc = bacc.Bacc("TRN2", target_bir_lowering=False, debug=False,
                   num_devices=NCORES)

    emb_d = nc.dram_tensor("emb", [V, H], F32, kind="ExternalInput")
    wih_d = nc.dram_tensor("wihT", [4, 128, GD], F32, kind="ExternalInput")
    whh_d = nc.dram_tensor("whhT", [4, 128, GD], F32, kind="ExternalInput")
    # streamed FC weights: [chunk][128][k*1024 + (hi=0/lo=512) + col]
    wfc_d = nc.dram_tensor("wfcs", [NQ, 128, 4 * 2 * CPAD], F16,
                           kind="ExternalInput")
    winit_d = nc.dram_tensor("winitT", [4, 128, 1024], F32, kind="ExternalInput")
    bg_d = nc.dram_tensor("bgate", [1, GD], F32, kind="ExternalInput")
    binit_d = nc.dram_tensor("binit", [1, 1024], F32, kind="ExternalInput")
    summ_d = nc.dram_tensor("summary", [B, H], F32, kind="ExternalInput")
    tok0_d = nc.dram_tensor("tok0", [B, 1], I32, kind="ExternalInput")
    qbase_d = nc.dram_tensor("qbase", [B, NQ * 8], F32, kind="ExternalInput")
    bsel_d = nc.dram_tensor("bsel", [64, NQ * B], F16, kind="ExternalInput")
    bpack_d = nc.dram_tensor("bpack", [64, 512], F16, kind="ExternalInput")

    out_d = nc.dram_tensor("logits", [out_slots, B, V], F16,
                           kind="ExternalOutput")

    with tile.TileContext(nc) as tc:
        import contextlib
        with contextlib.ExitStack() as ctx:
            const = ctx.enter_context(tc.tile_pool(name="const", bufs=1))
            work = ctx.enter_context(tc.tile_pool(name="work", bufs=1))
            hc = ctx.enter_context(tc.tile_pool(name="hc", bufs=2))
            small = ctx.enter_context(tc.tile_pool(name="small", bufs=2))
            stream = ctx.enter_context(tc.tile_pool(name="stream", bufs=5))
            lgout = ctx.enter_context(tc.tile_pool(name="lgout", bufs=3))
            ptr = ctx.enter_context(tc.tile_pool(name="ptr", bufs=1, space="PSUM"))
            pg = ctx.enter_context(tc.tile_pool(name="pg", bufs=1, space="PSUM"))
            pfc = ctx.enter_context(tc.tile_pool(name="pfc", bufs=3, space="PSUM"))

            # ---- constants / resident weights ----
            ident = const.tile([B, B], F32)
            make_identity(nc, ident[:])
            ones16 = const.tile([1, B], F16)
            nc.vector.memset(ones16[:], 1.0)
            qbase = const.tile([B, NQ * 8], F32)
            nc.sync.dma_start(qbase[:], qbase_d[:])

            wih = []
            whh = []
            for k in range(4):
                w = const.tile([128, GD], F32, tag=f"wih{k}")
                nc.sync.dma_start(w[:], wih_d[k])
                wih.append(w)
                w = const.tile([128, GD], F32, tag=f"whh{k}")
                nc.sync.dma_start(w[:], whh_d[k])
                whh.append(w)
            bg = const.tile([1, GD], F32)
            nc.sync.dma_start(bg[:], bg_d[:])
            binit = const.tile([1, 1024], F32)
            nc.sync.dma_start(binit[:], binit_d[:])
            onesf = const.tile([1, B], F32)
            nc.vector.memset(onesf[:], 1.0)
            bsel = const.tile([64, NQ * B], F16)
            nc.sync.dma_start(bsel[:], bsel_d[:])
            bpack = const.tile([64, 512], F16)
            nc.sync.dma_start(bpack[:], bpack_d[:])

            def transpose_to(src, dst_tile):
                """src SBUF [B, 512] f32 -> dst SBUF [128, 4*B] (k-packed)."""
                for k in range(4):
                    pt = ptr.tile([128, B], F32, tag="ptr")
                    nc.tensor.transpose(
                        out=pt[:], in_=src[:, k * 128:(k + 1) * 128],
                        identity=ident[:])
                    nc.scalar.copy(dst_tile[:, k * B:(k + 1) * B], pt[:])

            # ---- phase 0: h0/c0 from host-computed encoder mean ----
            summ = work.tile([B, H], F32, tag="gx")
            nc.sync.dma_start(summ[:], summ_d[:])
            sumT = work.tile([128, 4 * B], F32, tag="xT")
            transpose_to(summ, sumT)
            for n in range(2):
                ph = pfc.tile([B, 512], F32, tag="pfc")
                for k in range(4):
                    wi = work.tile([128, 1024], F32, tag="winit")
                    nc.sync.dma_start(wi[:], winit_d[k])
                    nc.tensor.matmul(
                        ph[:], lhsT=sumT[:, k * B:(k + 1) * B],
                        rhs=wi[:, n * 512:(n + 1) * 512],
                        start=(k == 0), stop=False)
                nc.tensor.matmul(
                    ph[:], lhsT=onesf[:], rhs=binit[:, n * 512:(n + 1) * 512],
                    start=False, stop=True)
                dst = hc.tile([B, H], F32, tag=("h" if n == 0 else "c"))
                nc.scalar.copy(dst[:], ph[:])
                if n == 0:
                    h_cur = dst
                else:
                    c_cur = dst

            hT = work.tile([128, 4 * B], F32, tag="hT")
            transpose_to(h_cur, hT)
            hhiT = work.tile([128, 4 * B], F16, tag="hhiT")
            nc.vector.tensor_copy(hhiT[:], hT[:])
            hres = work.tile([128, 4 * B], F32, tag="hres")
            nc.vector.tensor_sub(hres[:], hT[:], hhiT[:])
            hloT = work.tile([128, 4 * B], F16, tag="hloT")
            nc.vector.tensor_copy(hloT[:], hres[:])

            tok = small.tile([B, 1], I32, tag="tok")
            nc.sync.dma_start(tok[:], tok0_d[:])

            # ---- decode steps ----
            for t in range(nsteps):
                # x = emb[tok]; h@W_hh runs on PE while the gather lands
                gx = work.tile([B, H], F32, tag="gx")
                nc.gpsimd.indirect_dma_start(
                    out=gx[:], out_offset=None, in_=emb_d[:],
                    in_offset=bass.IndirectOffsetOnAxis(ap=tok[:, :1], axis=0))
                pgt = pg.tile([B, GD], F32, tag="pg")
                for n in range(4):
                    sl = slice(n * 512, (n + 1) * 512)
                    for k in range(4):
                        nc.tensor.matmul(
                            pgt[:, sl], lhsT=hT[:, k * B:(k + 1) * B],
                            rhs=whh[k][:, sl], start=(k == 0), stop=False)
                xT = work.tile([128, 4 * B], F32, tag="xT")
                transpose_to(gx, xT)
                for n in range(4):
                    sl = slice(n * 512, (n + 1) * 512)
                    for k in range(4):
                        nc.tensor.matmul(
                            pgt[:, sl], lhsT=xT[:, k * B:(k + 1) * B],
                            rhs=wih[k][:, sl], start=False, stop=False)
                    nc.tensor.matmul(
                        pgt[:, sl], lhsT=onesf[:], rhs=bg[:, sl],
                        start=False, stop=True)

                # pointwise LSTM
                sig_if = work.tile([B, 1024], F32, tag="sigif")
                nc.scalar.activation(sig_if[:], pgt[:, 0:1024], AF.Sigmoid)
                tng = work.tile([B, 512], F32, tag="tng")
                nc.scalar.activation(tng[:], pgt[:, 1024:1536], AF.Tanh)
                sgo = work.tile([B, 512], F32, tag="sgo")
                nc.scalar.activation(sgo[:], pgt[:, 1536:2048], AF.Sigmoid)

                t1 = work.tile([B, 512], F32, tag="t1")
                nc.vector.tensor_mul(t1[:], sig_if[:, 0:512], tng[:])
                t2 = work.tile([B, 512], F32, tag="t2")
                nc.vector.tensor_mul(t2[:], sig_if[:, 512:1024], c_cur[:])
                c_new = hc.tile([B, H], F32, tag="c")
                nc.vector.tensor_add(c_new[:], t2[:], t1[:])
                tc2 = work.tile([B, 512], F32, tag="tc2")
                nc.scalar.activation(tc2[:], c_new[:], AF.Tanh)
                h_new = hc.tile([B, H], F32, tag="h")
                nc.vector.tensor_mul(h_new[:], sgo[:], tc2[:])
                c_cur = c_new

                hT = work.tile([128, 4 * B], F32, tag="hT")
                transpose_to(h_new, hT)
                hhiT = work.tile([128, 4 * B], F16, tag="hhiT")
                nc.vector.tensor_copy(hhiT[:], hT[:])
                hres = work.tile([128, 4 * B], F32, tag="hres")
                nc.vector.tensor_sub(hres[:], hT[:], hhiT[:])
                hloT = work.tile([128, 4 * B], F16, tag="hloT")
                nc.vector.tensor_copy(hloT[:], hres[:])

                # FC over full vocab, streamed fp16 hi/lo 3-pass
                cands = small.tile([B, NQ * 8], F32, tag="cands")
                cidx = small.tile([B, NQ * 8], U32, tag="cidx")
                for q, (off, w) in enumerate(CHUNKS):
                    wt = stream.tile([128, 4 * 2 * CPAD], F16, tag="wt")
                    nc.sync.dma_start(wt[:], wfc_d[q])
                    pf = pfc.tile([B, 512], F32, tag="pfc")
                    for k in range(4):
                        nc.tensor.matmul(
                            pf[:, :w], lhsT=hhiT[:, k * B:(k + 1) * B],
                            rhs=wt[:, k * 1024:k * 1024 + w],
                            start=(k == 0), stop=False)
                    for k in range(4):
                        nc.tensor.matmul(
                            pf[:, :w], lhsT=hloT[:, k * B:(k + 1) * B],
                            rhs=wt[:, k * 1024:k * 1024 + w],
                            start=False, stop=False)
                        nc.tensor.matmul(
                            pf[:, :w], lhsT=hhiT[:, k * B:(k + 1) * B],
                            rhs=wt[:, k * 1024 + CPAD:k * 1024 + CPAD + w],
                            start=False, stop=False)
                    nc.tensor.matmul(
                        pf[:, :w], lhsT=bsel[:, q * B:(q + 1) * B],
                        rhs=bpack[:, :w], start=False, stop=True)
                    nc.vector.max(out=cands[:, q * 8:(q + 1) * 8], in_=pf[:, :w])
                    nc.vector.max_index(
                        out=cidx[:, q * 8:(q + 1) * 8],
                        in_max=cands[:, q * 8:(q + 1) * 8], in_values=pf[:, :w])
                    lg = lgout.tile([B, 512], F16, tag="lg")
                    nc.scalar.copy(lg[:, :w], pf[:, :w])
                    nc.sync.dma_start(out_d[t][:, off:off + w], lg[:, :w])

                # exact global argmax (first-occurrence tie-breaks)
                wv = small.tile([B, 8], F32, tag="wv")
                nc.vector.max(out=wv[:], in_=cands[:])
                msk = small.tile([B, NQ * 8], F32, tag="msk")
                nc.vector.tensor_scalar(
                    out=msk[:], in0=cands[:], scalar1=wv[:, 0:1], scalar2=None,
                    op0=OP.is_equal)
                idxf = small.tile([B, NQ * 8], F32, tag="idxf")
                nc.vector.tensor_copy(idxf[:], cidx[:])
                gidx = small.tile([B, NQ * 8], F32, tag="gidx")
                nc.vector.tensor_add(gidx[:], idxf[:], qbase[:])
                gneg = small.tile([B, NQ * 8], F32, tag="gneg")
                nc.vector.tensor_scalar(
                    out=gneg[:], in0=gidx[:], scalar1=-1.0, scalar2=48000.0,
                    op0=OP.mult, op1=OP.add)
                gsel = small.tile([B, NQ * 8], F32, tag="gsel")
                nc.vector.tensor_mul(gsel[:], msk[:], gneg[:])
                w2 = small.tile([B, 8], F32, tag="w2")
                nc.vector.max(out=w2[:], in_=gsel[:])
                tokf = small.tile([B, 1], F32, tag="tokf")
                nc.vector.tensor_scalar(
                    out=tokf[:], in0=w2[:, 0:1], scalar1=-1.0, scalar2=48000.0,
                    op0=OP.mult, op1=OP.add)
                tok = small.tile([B, 1], I32, tag="tok")
                nc.vector.tensor_copy(tok[:], tokf[:])

    nc.compile()
    return nc


def _prep_inputs(inputs):
    enc = np.asarray(inputs["encoder_outputs"], np.float32)
    captions = np.asarray(inputs["captions"])
    emb = np.ascontiguousarray(np.asarray(inputs["embedding"], np.float32))
    W_ih = np.asarray(inputs["W_ih"], np.float32)
    b_ih = np.asarray(inputs["b_ih"], np.float32)
    W_hh = np.asarray(inputs["W_hh"], np.float32)
    b_hh = np.asarray(inputs["b_hh"], np.float32)
    W_fc = np.asarray(inputs["W_fc"], np.float32)
    b_fc = np.asarray(inputs["b_fc"], np.float32)
    W_init_h = np.asarray(inputs["W_init_h"], np.float32)
    b_init_h = np.asarray(inputs["b_init_h"], np.float32)
    W_init_c = np.asarray(inputs["W_init_c"], np.float32)
    b_init_c = np.asarray(inputs["b_init_c"], np.float32)

    wihT = np.ascontiguousarray(W_ih.T.reshape(4, 128, GD))
    whhT = np.ascontiguousarray(W_hh.T.reshape(4, 128, GD))
    winitT = np.ascontiguousarray(
        np.concatenate([W_init_h, W_init_c], axis=0).T.reshape(4, 128, 1024))
    binit = np.concatenate([b_init_h, b_init_c]).reshape(1, 1024)
    bgate = (b_ih + b_hh).reshape(1, GD)
    summary = enc.mean(axis=1).astype(np.float32)
    tok0 = np.ascontiguousarray(captions[:, 0].astype(np.int32).reshape(B, 1))

    WT = W_fc.T.astype(np.float32)          # [512, 32000]
    Whi = WT.astype(np.float16)
    Wlo = (WT - Whi.astype(np.float32)).astype(np.float16)
    bfc16 = b_fc.astype(np.float16)
    wfcs = np.zeros((NQ, 128, 4 * 2 * CPAD), np.float16)
    for q, (off, w) in enumerate(CHUNKS):
        for k in range(4):
            wfcs[q, :, k * 1024:k * 1024 + w] = \
                Whi[k * 128:(k + 1) * 128, off:off + w]
            wfcs[q, :, k * 1024 + CPAD:k * 1024 + CPAD + w] = \
                Wlo[k * 128:(k + 1) * 128, off:off + w]
    bsel = np.zeros((64, NQ * B), np.float16)
    for q in range(NQ):
        bsel[q, q * B:(q + 1) * B] = 1.0
    bpack = np.zeros((64, 512), np.float16)
    for q, (off, w) in enumerate(CHUNKS):
        bpack[q, :w] = bfc16[off:off + w]

    qbase = np.zeros((B, NQ * 8), np.float32)
    for q, (off, _w) in enumerate(CHUNKS):
        qbase[:, q * 8:(q + 1) * 8] = float(off)

    in_map = {
        "emb": emb,
        "wihT": wihT,
        "whhT": whhT,
        "wfcs": wfcs,
        "winitT": winitT,
        "bgate": bgate,
        "binit": binit,
        "summary": summary,
        "tok0": tok0,
        "qbase": qbase,
        "bsel": bsel,
        "bpack": bpack,
    }
    return [in_map for _ in range(NCORES)]


def kernel(**inputs) -> np.ndarray:
    if "nc" not in _CACHE:
        _CACHE["nc"] = _build_nc()
    nc = _CACHE["nc"]
    in_maps = _prep_inputs(inputs)
    res = run_bass_kernel_spmd(nc, in_maps, list(range(NCORES)))
    out = np.zeros((B, T, V), np.float32)
    lg = res.results[0]["logits"][:NSTEPS].astype(np.float32)  # [31, 64, V]
    out[:, 1:, :] = lg.transpose(1, 0, 2)
    return out


# revision 6
# speedup vs baseline: 3860.5356x; 1.2622x over previous
"""Trainium2 Bass kernel for nn_DecoderLSTM_noAttention — collective-free.

Strategy: every core runs the FULL greedy decode independently (replicated
compute, zero cross-core communication — collectives cost ~1 network RTT each
under the axon tunnel and dominated the old kernel's runtime).

Per step:
- indirect-gather embedding rows for the current tokens [64, 512]
- transpose x on PE; gates = x@W_ih.T + h@W_hh.T + (b_ih+b_hh) in fp32 on PE
- pointwise LSTM via activation tables (same numerics as before)
- FC over the FULL vocab with fp16 hi/lo 3-pass matmuls (error ~1e-7,
  preserves exact argmax vs the fp32 reference); W_fc streamed from DRAM
  (66 MB/step) since the full-vocab weights don't fit in SBUF
- per-chunk DVE max/max_index straight from PSUM; exact first-occurrence
  tie-break combine; next token fully local
- logits written out as fp16 (output tolerance is 2e-2; fp16 adds ~5e-4)

Output: each core writes identical [31, 64, 32000] fp16 logits; the host uses
core 0's copy.
"""
import numpy as np

import concourse.bass as bass
import concourse.bacc as bacc
import concourse.tile as tile
from concourse import mybir
from concourse.bass_utils import run_bass_kernel_spmd
from concourse.masks import make_identity

F32 = mybir.dt.float32
F16 = mybir.dt.float16
I32 = mybir.dt.int32
U32 = mybir.dt.uint32
AF = mybir.ActivationFunctionType
OP = mybir.AluOpType

B = 64
H = 512
V = 32000
T = 32
NPIX = 196
NCORES = 8
NSTEPS = T - 1
GD = 2048

# FC chunking over the full vocab: 62x512 + 1x256
CHUNKS = [(q * 512, 512) for q in range(62)] + [(31744, 256)]
NQ = len(CHUNKS)
CPAD = 512  # per-chunk padded width in the streamed weight layout

_CACHE = {}


def _build_nc(nsteps=NSTEPS, out_slots=NSTEPS):
    nc = bacc.Bacc("TRN2", target_bir_lowering=False, debug=False,
                   num_devices=NCORES)

    emb_d = nc.dram_tensor("emb", [V, H], F32, kind="ExternalInput")
    wih_d = nc.dram_tensor("wihT", [4, 128, GD], F32, kind="ExternalInput")
    whh_d = nc.dram_tensor("whhT", [4, 128, GD], F32, kind="ExternalInput")
    # streamed FC weights: [chunk][128][k*1024 + (hi=0/lo=512) + col]
    wfc_d = nc.dram_tensor("wfcs", [NQ, 128, 4 * 2 * CPAD], F16,
                           kind="ExternalInput")
    winit_d = nc.dram_tensor("winitT", [4, 128, 1024], F32, kind="ExternalInput")
    bg_d = nc.dram_tensor("bgate", [1, GD], F32, kind="ExternalInput")
    binit_d = nc.dram_tensor("binit", [1, 1024], F32, kind="ExternalInput")
    summ_d = nc.dram_tensor("summary", [B, H], F32, kind="ExternalInput")
    tok0_d = nc.dram_tensor("tok0", [B, 1], I32, kind="ExternalInput")
    qbase_d = nc.dram_tensor("qbase", [B, NQ * 8], F32, kind="ExternalInput")
    bsel_d = nc.dram_tensor("bsel", [64, NQ * B], F16, kind="ExternalInput")
    bpack_d = nc.dram_tensor("bpack", [64, 512], F16, kind="ExternalInput")

    out_d = nc.dram_tensor("logits", [out_slots, B, V], F16,
                           kind="ExternalOutput")

    with tile.TileContext(nc) as tc:
        import contextlib
        with contextlib.ExitStack() as ctx:
            const = ctx.enter_context(tc.tile_pool(name="const", bufs=1))
            work = ctx.enter_context(tc.tile_pool(name="work", bufs=1))
            hc = ctx.enter_context(tc.tile_pool(name="hc", bufs=2))
            small = ctx.enter_context(tc.tile_pool(name="small", bufs=2))
            stream = ctx.enter_context(tc.tile_pool(name="stream", bufs=5))
            lgout = ctx.enter_context(tc.tile_pool(name="lgout", bufs=3))
            ptr = ctx.enter_context(tc.tile_pool(name="ptr", bufs=1, space="PSUM"))
            pg = ctx.enter_context(tc.tile_pool(name="pg", bufs=1, space="PSUM"))
            pfc = ctx.enter_context(tc.tile_pool(name="pfc", bufs=3, space="PSUM"))

            # ---- constants / resident weights ----
            ident = const.tile([B, B], F32)
            make_identity(nc, ident[:])
            ones16 = const.tile([1, B], F16)
            nc.vector.memset(ones16[:], 1.0)
            qbase = const.tile([B, NQ * 8], F32)
            nc.sync.dma_start(qbase[:], qbase_d[:])

            wih = []
            whh = []
            for k in range(4):
                w = const.tile([128, GD], F32, tag=f"wih{k}")
                nc.sync.dma_start(w[:], wih_d[k])
                wih.append(w)
                w = const.tile([128, GD], F32, tag=f"whh{k}")
                nc.sync.dma_start(w[:], whh_d[k])
                whh.append(w)
            bg = const.tile([1, GD], F32)
            nc.sync.dma_start(bg[:], bg_d[:])
            binit = const.tile([1, 1024], F32)
            nc.sync.dma_start(binit[:], binit_d[:])
            onesf = const.tile([1, B], F32)
            nc.vector.memset(onesf[:], 1.0)
            bsel = const.tile([64, NQ * B], F16)
            nc.sync.dma_start(bsel[:], bsel_d[:])
            bpack = const.tile([64, 512], F16)
            nc.sync.dma_start(bpack[:], bpack_d[:])

            def transpose_to(src, dst_tile):
                """src SBUF [B, 512] f32 -> dst SBUF [128, 4*B] (k-packed)."""
                for k in range(4):
                    pt = ptr.tile([128, B], F32, tag="ptr")
                    nc.tensor.transpose(
                        out=pt[:], in_=src[:, k * 128:(k + 1) * 128],
                        identity=ident[:])
                    nc.scalar.copy(dst_tile[:, k * B:(k + 1) * B], pt[:])

            # ---- phase 0: h0/c0 from host-computed encoder mean ----
            summ = work.tile([B, H], F32, tag="gx")
            nc.sync.dma_start(summ[:], summ_d[:])
            sumT = work.tile([128, 4 * B], F32, tag="xT")
            transpose_to(summ, sumT)
            for n in range(2):
                ph = pfc.tile([B, 512], F32, tag="pfc")
                for k in range(4):
                    wi = work.tile([128, 1024], F32, tag="winit")
                    nc.sync.dma_start(wi[:], winit_d[k])
                    nc.tensor.matmul(
                        ph[:], lhsT=sumT[:, k * B:(k + 1) * B],
                        rhs=wi[:, n * 512:(n + 1) * 512],
                        start=(k == 0), stop=False)
                nc.tensor.matmul(
                    ph[:], lhsT=onesf[:], rhs=binit[:, n * 512:(n + 1) * 512],
                    start=False, stop=True)
                dst = hc.tile([B, H], F32, tag=("h" if n == 0 else "c"))
                nc.scalar.copy(dst[:], ph[:])
                if n == 0:
                    h_cur = dst
                else:
                    c_cur = dst

            hT = work.tile([128, 4 * B], F32, tag="hT")
            transpose_to(h_cur, hT)
            hhiT = work.tile([128, 4 * B], F16, tag="hhiT")
            nc.vector.tensor_copy(hhiT[:], hT[:])
            hres = work.tile([128, 4 * B], F32, tag="hres")
            nc.vector.tensor_sub(hres[:], hT[:], hhiT[:])
            hloT = work.tile([128, 4 * B], F16, tag="hloT")
            nc.vector.tensor_copy(hloT[:], hres[:])

            tok = small.tile([B, 1], I32, tag="tok")
            nc.sync.dma_start(tok[:], tok0_d[:])

            # ---- decode steps ----
            for t in range(nsteps):
                # x = emb[tok]; h@W_hh runs on PE while the gather lands
                gx = work.tile([B, H], F32, tag="gx")
                nc.gpsimd.indirect_dma_start(
                    out=gx[:], out_offset=None, in_=emb_d[:],
                    in_offset=bass.IndirectOffsetOnAxis(ap=tok[:, :1], axis=0))
                pgt = pg.tile([B, GD], F32, tag="pg")
                for n in range(4):
                    sl = slice(n * 512, (n + 1) * 512)
                    for k in range(4):
                        nc.tensor.matmul(
                            pgt[:, sl], lhsT=hT[:, k * B:(k + 1) * B],
                            rhs=whh[k][:, sl], start=(k == 0), stop=False)
                xT = work.tile([128, 4 * B], F32, tag="xT")
                transpose_to(gx, xT)
                for n in range(4):
                    sl = slice(n * 512, (n + 1) * 512)
                    for k in range(4):
                        nc.tensor.matmul(
                            pgt[:, sl], lhsT=xT[:, k * B:(k + 1) * B],
                            rhs=wih[k][:, sl], start=False, stop=False)
                    nc.tensor.matmul(
                        pgt[:, sl], lhsT=onesf[:], rhs=bg[:, sl],
                        start=False, stop=True)

                # pointwise LSTM
                sig_if = work.tile([B, 1024], F32, tag="sigif")
                nc.scalar.activation(sig_if[:], pgt[:, 0:1024], AF.Sigmoid)
                tng = work.tile([B, 512], F32, tag="tng")
                nc.scalar.activation(tng[:], pgt[:, 1024:1536], AF.Tanh)
                sgo = work.tile([B, 512], F32, tag="sgo")
                nc.scalar.activation(sgo[:], pgt[:, 1536:2048], AF.Sigmoid)

                t1 = work.tile([B, 512], F32, tag="t1")
                nc.vector.tensor_mul(t1[:], sig_if[:, 0:512], tng[:])
                t2 = work.tile([B, 512], F32, tag="t2")
                nc.vector.tensor_mul(t2[:], sig_if[:, 512:1024], c_cur[:])
                c_new = hc.tile([B, H], F32, tag="c")
                nc.vector.tensor_add(c_new[:], t2[:], t1[:])
                tc2 = work.tile([B, 512], F32, tag="tc2")
                nc.scalar.activation(tc2[:], c_new[:], AF.Tanh)
                h_new = hc.tile([B, H], F32, tag="h")
                nc.vector.tensor_mul(h_new[:], sgo[:], tc2[:])
                c_cur = c_new

                hT = work.tile([128, 4 * B], F32, tag="hT")
                transpose_to(h_new, hT)
                hhiT = work.tile([128, 4 * B], F16, tag="hhiT")
                nc.vector.tensor_copy(hhiT[:], hT[:])
                hres = work.tile([128, 4 * B], F32, tag="hres")
                nc.vector.tensor_sub(hres[:], hT[:], hhiT[:])
                hloT = work.tile([128, 4 * B], F16, tag="hloT")
                nc.vector.tensor_copy(hloT[:], hres[:])

                # FC over full vocab, streamed fp16 hi/lo 3-pass
                cands = small.tile([B, NQ * 8], F32, tag="cands")
                cidx = small.tile([B, NQ * 8], U32, tag="cidx")
                for q, (off, w) in enumerate(CHUNKS):
                    wt = stream.tile([128, 4 * 2 * CPAD], F16, tag="wt")
                    # alternate issue rings (SP / ACT HWDGE) for DMA overlap
                    if q % 2 == 0:
                        nc.sync.dma_start(wt[:], wfc_d[q])
                    else:
                        nc.scalar.dma_start(wt[:], wfc_d[q])
                    pf = pfc.tile([B, 512], F32, tag="pfc")
                    for k in range(4):
                        nc.tensor.matmul(
                            pf[:, :w], lhsT=hhiT[:, k * B:(k + 1) * B],
                            rhs=wt[:, k * 1024:k * 1024 + w],
                            start=(k == 0), stop=False)
                    for k in range(4):
                        nc.tensor.matmul(
                            pf[:, :w], lhsT=hloT[:, k * B:(k + 1) * B],
                            rhs=wt[:, k * 1024:k * 1024 + w],
                            start=False, stop=False)
                        nc.tensor.matmul(
                            pf[:, :w], lhsT=hhiT[:, k * B:(k + 1) * B],
                            rhs=wt[:, k * 1024 + CPAD:k * 1024 + CPAD + w],
                            start=False, stop=False)
                    nc.tensor.matmul(
                        pf[:, :w], lhsT=bsel[:, q * B:(q + 1) * B],
                        rhs=bpack[:, :w], start=False, stop=True)
                    nc.vector.max(out=cands[:, q * 8:(q + 1) * 8], in_=pf[:, :w])
                    nc.vector.max_index(
                        out=cidx[:, q * 8:(q + 1) * 8],
                        in_max=cands[:, q * 8:(q + 1) * 8], in_values=pf[:, :w])
                    lg = lgout.tile([B, 512], F16, tag="lg")
                    nc.scalar.copy(lg[:, :w], pf[:, :w])
                    nc.sync.dma_start(out_d[t][:, off:off + w], lg[:, :w])

                # exact global argmax (first-occurrence tie-breaks)
                wv = small.tile([B, 8], F32, tag="wv")
                nc.vector.max(out=wv[:], in_=cands[:])
                msk = small.tile([B, NQ * 8], F32, tag="msk")
                nc.vector.tensor_scalar(
                    out=msk[:], in0=cands[:], scalar1=wv[:, 0:1], scalar2=None,
                    op0=OP.is_equal)
                idxf = small.tile([B, NQ * 8], F32, tag="idxf")
                nc.vector.tensor_copy(idxf[:], cidx[:])
                gidx = small.tile([B, NQ * 8], F32, tag="gidx")
                nc.vector.tensor_add(gidx[:], idxf[:], qbase[:])
                gneg = small.tile([B, NQ * 8], F32, tag="gneg")
                nc.vector.tensor_scalar(
                    out=gneg[:], in0=gidx[:], scalar1=-1.0, scalar2=48000.0,
                    op0=OP.mult, op1=OP.add)
                gsel = small.tile([B, NQ * 8], F32, tag="gsel")
                nc.vector.tensor_mul(gsel[:], msk[:], gneg[:])
                w2 = small.tile([B, 8], F32, tag="w2")
                nc.vector.max(out=w2[:], in_=gsel[:])
                tokf = small.tile([B, 1], F32, tag="tokf")
                nc.vector.tensor_scalar(
                    out=tokf[:], in0=w2[:, 0:1], scalar1=-1.0, scalar2=48000.0,
                    op0=OP.mult, op1=OP.add)
                tok = small.tile([B, 1], I32, tag="tok")
                nc.vector.tensor_copy(tok[:], tokf[:])

    nc.compile()
    return nc


def _prep_inputs(inputs):
    enc = np.asarray(inputs["encoder_outputs"], np.float32)
    captions = np.asarray(inputs["captions"])
    emb = np.ascontiguousarray(np.asarray(inputs["embedding"], np.float32))
    W_ih = np.asarray(inputs["W_ih"], np.float32)
    b_ih = np.asarray(inputs["b_ih"], np.float32)
    W_hh = np.asarray(inputs["W_hh"], np.float32)
    b_hh = np.asarray(inputs["b_hh"], np.float32)
    W_fc = np.asarray(inputs["W_fc"], np.float32)
    b_fc = np.asarray(inputs["b_fc"], np.float32)
    W_init_h = np.asarray(inputs["W_init_h"], np.float32)
    b_init_h = np.asarray(inputs["b_init_h"], np.float32)
    W_init_c = np.asarray(inputs["W_init_c"], np.float32)
    b_init_c = np.asarray(inputs["b_init_c"], np.float32)

    wihT = np.ascontiguousarray(W_ih.T.reshape(4, 128, GD))
    whhT = np.ascontiguousarray(W_hh.T.reshape(4, 128, GD))
    winitT = np.ascontiguousarray(
        np.concatenate([W_init_h, W_init_c], axis=0).T.reshape(4, 128, 1024))
    binit = np.concatenate([b_init_h, b_init_c]).reshape(1, 1024)
    bgate = (b_ih + b_hh).reshape(1, GD)
    summary = enc.mean(axis=1).astype(np.float32)
    tok0 = np.ascontiguousarray(captions[:, 0].astype(np.int32).reshape(B, 1))

    WT = W_fc.T.astype(np.float32)          # [512, 32000]
    Whi = WT.astype(np.float16)
    Wlo = (WT - Whi.astype(np.float32)).astype(np.float16)
    bfc16 = b_fc.astype(np.float16)
    wfcs = np.zeros((NQ, 128, 4 * 2 * CPAD), np.float16)
    for q, (off, w) in enumerate(CHUNKS):
        for k in range(4):
            wfcs[q, :, k * 1024:k * 1024 + w] = \
                Whi[k * 128:(k + 1) * 128, off:off + w]
            wfcs[q, :, k * 1024 + CPAD:k * 1024 + CPAD + w] = \
                Wlo[k * 128:(k + 1) * 128, off:off + w]
    bsel = np.zeros((64, NQ * B), np.float16)
    for q in range(NQ):
        bsel[q, q * B:(q + 1) * B] = 1.0
    bpack = np.zeros((64, 512), np.float16)
    for q, (off, w) in enumerate(CHUNKS):
        bpack[q, :w] = bfc16[off:off + w]

    qbase = np.zeros((B, NQ * 8), np.float32)
    for q, (off, _w) in enumerate(CHUNKS):
        qbase[:, q * 8:(q + 1) * 8] = float(off)

    in_map = {
        "emb": emb,
        "wihT": wihT,
        "whhT": whhT,
        "wfcs": wfcs,
        "winitT": winitT,
        "bgate": bgate,
        "binit": binit,
        "summary": summary,
        "tok0": tok0,
        "qbase": qbase,
        "bsel": bsel,
        "bpack": bpack,
    }
    return [in_map for _ in range(NCORES)]


def kernel(**inputs) -> np.ndarray:
    if "nc" not in _CACHE:
        _CACHE["nc"] = _build_nc()
    nc = _CACHE["nc"]
    in_maps = _prep_inputs(inputs)
    res = run_bass_kernel_spmd(nc, in_maps, list(range(NCORES)))
    out = np.zeros((B, T, V), np.float32)
    lg = res.results[0]["logits"][:NSTEPS].astype(np.float32)  # [31, 64, V]
    out[:, 1:, :] = lg.transpose(1, 0, 2)
    return out
